# revision 18
# baseline (speedup 1.0000x reference)
"""Trainium2 Bass kernel for nn_BilinearPairedLayer.

out[b,i,j,o] = celu(zl[b,i] @ fc_l_W^T + fc_l_b) @ W[o] @ celu(zr[b,j] @ fc_r_W^T + fc_r_b) + bb[o]

with context-3 pairing:
  zl = [x_l, shift_fwd(x_l,1), shift_bwd(x_l,1)]   (192 features)
  zr = [x_l, shift_bwd(x_r,1), shift_fwd(x_r,1)]   (faithful torch-source bug: x_l first)

Shapes: B=2, N=512, n_in=64, H=128, n_out=8 -> out [2,512,512,8] f32.

Sharding: 8 cores = (b in {0,1}) x (j-chunk in {0..3} of 128 columns).
Each core computes out[b, :, j0:j0+128, :] = [512, 128, 8] = 2MB.

Per-core dataflow (contraction dims pre-transposed onto partitions host-side):
  A1 [128,642]  = frT | xljT | xrhT        (hr operands, f32r)
  A2 [128,898]  = flT | xlhT               (hl operands, f32r)
  Wt [128,1024] = per-o transposed bilinear_W  (f32r)
  Cb [128,2]    = fc_l_b | fc_r_b          (f32)
  Ct [128,512]  = btile                    (f32)

  0. f32r spin matmuls (DVE memset+cast producer) warm the PE HAM
     clock-gate (1.2 -> 2.4 GHz) during the input-DMA wait.
  1. hrT[g,j] = celu(sum_c frT_c.T @ {xljT, xrhT +-1} + br)   3 matmuls, n=128
  2. hlT[h,i] = celu(sum_c flT_c.T @ xlhT_shift_c + bl)       3 matmuls, n=512
     celu(x) = max(x+b, min(exp(x+b) - 1, 0)) — exp overflow is clamped by
     the min, so no pre-clamp needed.
  3. v2[h,(j,o)] : per o: WT_o.T @ hrT -> psum [h, o*128+j]; one strided
     ACT cast per 4 o's into (j*8+o) layout.
  4. out[i,(j,o)] = hlT_chunk.T @ v2_half + btile  (8 matmuls [128k,128m,512n]).
     Evictions alternate DVE (fused +btile) and ACT copy + GpSimd +btile.
  5. DMA out per (i-chunk, half): [128, 512] contiguous 2KB rows.

walrus's per-instruction HW structs carry at most ONE sync wait; a post-pass
splits multi-wait instructions into single-wait EventSemaphore predecessors.
"""

import os

import numpy as np

import concourse.bass as bass
import concourse.mybir as mybir
import concourse.tile as tile
from concourse.bass_utils import run_bass_kernel_spmd

F32 = mybir.dt.float32
F32R = mybir.dt.float32r

B = 2
N = 512
NIN = 64
H = 128
O = 8
JC = 128  # j-chunk per core
N_CORES = 8

USE_F32R = os.environ.get("BK_F32R", "1") == "1"
N_SPIN = int(os.environ.get("BK_SPIN", "10"))

# packed-A1 column offsets (hr operands)
_FR0 = 0            # frT   [3*128]
_XLJ = 384          # xljT  [128]
_XRH = 512          # xrhT  [130]
_A1W = 642
# packed-A2 column offsets (hl operands)
_FL0 = 0            # flT   [3*128]
_XLH = 384          # xlhT  [514]
_A2W = 898


def build_nc():
    nc = bass.Bass("TRN2")
    dt_mm = F32R if USE_F32R else F32

    A1 = nc.dram_tensor("A1", [128, _A1W], dt_mm, kind="ExternalInput")
    A2 = nc.dram_tensor("A2", [128, _A2W], dt_mm, kind="ExternalInput")
    Wt = nc.dram_tensor("Wt", [128, O * H], dt_mm, kind="ExternalInput")
    Cb = nc.dram_tensor("Cb", [128, 2], F32, kind="ExternalInput")
    Ct = nc.dram_tensor("Ct", [128, 512], F32, kind="ExternalInput")
    out_d = nc.dram_tensor("out", [N, JC * O], F32, kind="ExternalOutput")

    with tile.TileContext(nc) as tc:
        with (
            tc.tile_pool(name="persist", bufs=1) as pp,
            tc.tile_pool(name="scratch", bufs=2) as sp,
            tc.tile_pool(name="outbuf", bufs=8) as op,
            tc.tile_pool(name="ps_l1", bufs=1, space="PSUM") as ps_l1,
            tc.tile_pool(name="ps_v", bufs=2, space="PSUM") as ps_v,
            tc.tile_pool(name="ps_main", bufs=4, space="PSUM") as ps_main,
        ):
            # Early no-dep ACT op so the lazy PWP activation-table load
            # (~1.3us) happens during the input-DMA wait, not on the
            # critical path before the first real activation.
            td = pp.tile([1, 2], F32, name="td")
            nc.vector.memset(td[0:1, 0:1], 0.0)
            nc.scalar.activation(td[0:1, 1:2], td[0:1, 0:1],
                                 mybir.ActivationFunctionType.Exp)

            A1_sb = pp.tile([128, _A1W], dt_mm, name="A1_sb")
            A2_sb = pp.tile([128, _A2W], dt_mm, name="A2_sb")
            W_sb = pp.tile([128, O * H], dt_mm, name="W_sb")
            Cb_sb = pp.tile([128, 2], F32, name="Cb_sb")
            Ct_sb = pp.tile([128, 512], F32, name="Ct_sb")
            nc.sync.dma_start(A1_sb[:], A1[:])
            nc.sync.dma_start(Cb_sb[:], Cb[:])
            nc.sync.dma_start(W_sb[:], Wt[:])
            nc.sync.dma_start(A2_sb[:], A2[:])
            nc.sync.dma_start(Ct_sb[:], Ct[:])

            bl_ap = Cb_sb[:, 0:1]
            br_ap = Cb_sb[:, 1:2]
            btile = Ct_sb[:]

            def celu_from_psum(psum, bias_ap, width, tag):
                """h = celu(psum + bias) = max(pre, min(exp(pre)-1, 0))."""
                pre = sp.tile([128, width], F32, name=f"pre_{tag}")
                e = sp.tile([128, width], F32, name=f"e_{tag}")
                h = pp.tile([128, width], dt_mm, name=f"h_{tag}")
                nc.scalar.activation(
                    pre[:], psum, mybir.ActivationFunctionType.Identity,
                    bias=bias_ap, scale=1.0,
                )
                nc.scalar.activation(
                    e[:], psum, mybir.ActivationFunctionType.Exp,
                    bias=bias_ap, scale=1.0,
                )
                # s = min(e - 1, 0)   (fused; overflow in e is clamped here)
                nc.vector.tensor_scalar(
                    e[:], e[:], -1.0, 0.0,
                    mybir.AluOpType.add, mybir.AluOpType.min,
                )
                nc.vector.tensor_tensor(h[:], pre[:], e[:], mybir.AluOpType.max)
                return h

            # ---- layer 1: hrT [128(g), JC] ----
            ps_hr = ps_l1.tile([128, JC], F32, name="ps_hr")
            rhs_r = [
                A1_sb[:, _XLJ:_XLJ + JC],           # x_l[j]
                A1_sb[:, _XRH + 2:_XRH + 2 + JC],   # x_r[j+1] (bwd)
                A1_sb[:, _XRH:_XRH + JC],           # x_r[j-1] (fwd)
            ]
            for c in range(3):
                nc.tensor.matmul(
                    ps_hr[:], A1_sb[:, _FR0 + c * H:_FR0 + (c + 1) * H], rhs_r[c],
                    start=(c == 0), stop=(c == 2),
                )
            hrT = celu_from_psum(ps_hr[:], br_ap, JC, "hr")

            # ---- layer 1: hlT [128(h), N] ----
            ps_hl = ps_l1.tile([128, N], F32, name="ps_hl")
            rhs_l = [
                A2_sb[:, _XLH + 1:_XLH + 1 + N],    # x_l[i]
                A2_sb[:, _XLH:_XLH + N],            # x_l[i-1] (fwd)
                A2_sb[:, _XLH + 2:_XLH + 2 + N],    # x_l[i+1] (bwd)
            ]
            for c in range(3):
                nc.tensor.matmul(
                    ps_hl[:], A2_sb[:, _FL0 + c * H:_FL0 + (c + 1) * H], rhs_l[c],
                    start=(c == 0), stop=(c == 2),
                )
            hlT = celu_from_psum(ps_hl[:], bl_ap, N, "hl")

            # ---- v2[h, (j,o)] ----
            v2 = pp.tile([128, JC, O], dt_mm, name="v2")
            for og in range(2):
                ps_vo = ps_v.tile([128, 512], F32, name="ps_vo")
                for ol in range(4):
                    o = og * 4 + ol
                    nc.tensor.matmul(
                        ps_vo[:, ol * JC:(ol + 1) * JC],
                        W_sb[:, o * H:(o + 1) * H], hrT[:],
                        start=True, stop=True,
                    )
                # strided cast: psum col (ol*128+j) -> v2[:, j, og*4+ol]
                # (one on DVE, one on ACT to balance engine load)
                cast_eng = nc.vector.tensor_copy if og == 0 else nc.scalar.copy
                cast_eng(
                    v2[:, :, og * 4:(og + 1) * 4],
                    ps_vo[:].rearrange("p (o j) -> p j o", o=4),
                )
            v2f = v2[:].rearrange("p j o -> p (j o)")

            # ---- main: out[i, (j,o)] ----
            # First 3 chunks evict via ACT copy + GpSimd bias-add (idle
            # engines), last 5 via DVE fused add so the tail chunk's
            # DMA isn't gated on the slower GpSimd chain.
            for chunk in range(8):
                ic, half = chunk // 2, chunk % 2
                ps_m = ps_main.tile([128, 512], F32, name="ps_m")
                out_sb = op.tile([128, 512], F32, name="out_sb")
                nc.tensor.matmul(
                    ps_m[:],
                    hlT[:, ic * 128:(ic + 1) * 128],
                    v2f[:, half * 512:(half + 1) * 512],
                    start=True, stop=True,
                )
                if chunk < 3:
                    nc.scalar.copy(out_sb[:], ps_m[:])
                    nc.gpsimd.tensor_tensor(
                        out_sb[:], out_sb[:], btile, mybir.AluOpType.add)
                else:
                    nc.vector.tensor_tensor(
                        out_sb[:], ps_m[:], btile, mybir.AluOpType.add)
                nc.sync.dma_start(
                    out_d[ic * 128:(ic + 1) * 128,
                          half * 512:(half + 1) * 512],
                    out_sb[:])

    _legalize_waits(nc)
    return nc


def _legalize_waits(nc):
    """walrus's per-instruction HW structs carry at most ONE sync wait.
    Split any instruction with >1 on_wait into same-engine single-wait
    EventSemaphore predecessors (engine executes them in program order)."""
    n = 0
    for bb in nc.main_func.blocks:
        insts = list(bb.instructions)
        out = []
        for ins in insts:
            si = ins.sync_info
            waits = list(si.on_wait) if si and si.on_wait else []
            if len(waits) > 1:
                for w in waits[:-1]:
                    n += 1
                    out.append(mybir.InstEventSemaphore(
                        name=f"wait-split-{n}",
                        opcode="EventSemaphore",
                        engine=ins.engine,
                        ins=[], outs=[],
                        sync_info=mybir.SyncInfo(on_wait=[w], on_update=[]),
                    ))
                si.on_wait = [waits[-1]]
            out.append(ins)
        if n:
            bb.instructions = out
    return nc


_NC_CACHE = None


def _get_nc():
    global _NC_CACHE
    if _NC_CACHE is None:
        _NC_CACHE = build_nc()
    return _NC_CACHE


def _prep_core_inputs(x_l, x_r, fc_l_W, fc_l_b, fc_r_W, fc_r_b, bilinear_W, bilinear_b):
    """Host-side sharding: build the 8 per-core input dicts."""
    f32 = np.float32
    x_l = np.ascontiguousarray(x_l, f32)
    x_r = np.ascontiguousarray(x_r, f32)

    # fc weights: [H,192] -> transposed per 64-chunk, zero-padded to 128 rows
    def fcT(w):
        out = np.zeros((128, 3 * H), f32)
        for c in range(3):
            out[:NIN, c * H:(c + 1) * H] = w[:, c * NIN:(c + 1) * NIN].T
        return out

    flT = fcT(np.asarray(fc_l_W, f32))
    frT = fcT(np.asarray(fc_r_W, f32))
    # WT[g, o*H + h] = W[o, h, g]
    WT = np.ascontiguousarray(
        np.asarray(bilinear_W, f32).transpose(2, 0, 1).reshape(128, O * H))
    Cb = np.zeros((128, 2), f32)
    Cb[:, 0] = np.asarray(fc_l_b, f32)
    Cb[:, 1] = np.asarray(fc_r_b, f32)
    Ct = np.ascontiguousarray(np.broadcast_to(
        np.tile(np.asarray(bilinear_b, f32), 512 // O), (128, 512)))

    A2 = np.zeros((128, _A2W), f32)
    A2[:, _FL0:_FL0 + 3 * H] = flT

    in_maps = []
    for core in range(N_CORES):
        b, jg = core // 4, core % 4
        j0 = jg * JC
        A1 = np.zeros((128, _A1W), f32)
        A1[:, _FR0:_FR0 + 3 * H] = frT
        A1[:NIN, _XLJ:_XLJ + JC] = x_l[b, j0:j0 + JC].T
        # xrhT: col t = x_r[b, j0-1+t], zero-padded at global edges
        lo = max(j0 - 1, 0)
        hi = min(j0 + JC + 1, N)
        A1[:NIN, _XRH + lo - (j0 - 1):_XRH + hi - (j0 - 1)] = x_r[b, lo:hi].T
        A2b = A2.copy()
        # xlhT: col t = x_l[b, t-1], zeros at t=0 and t=N+1
        A2b[:NIN, _XLH + 1:_XLH + 1 + N] = x_l[b].T
        in_maps.append({"A1": A1, "A2": A2b, "Wt": WT, "Cb": Cb, "Ct": Ct})
    return in_maps


def _run(inputs, trace=False, **kw):
    nc = _get_nc()
    in_maps = _prep_core_inputs(**inputs)
    res = run_bass_kernel_spmd(
        nc, in_maps, core_ids=list(range(N_CORES)), trace=trace, **kw)
    out = np.empty((B, N, N, O), np.float32)
    for core in range(N_CORES):
        b, jg = core // 4, core % 4
        j0 = jg * JC
        out[b, :, j0:j0 + JC, :] = res.results[core]["out"].reshape(N, JC, O)
    return out, res


def kernel(**inputs):
    out, _ = _run(inputs, trace=False)
    return out


# revision 19
# speedup vs baseline: 1.1888x; 1.1888x over previous
"""Trainium2 Bass kernel for nn_BilinearPairedLayer.

out[b,i,j,o] = celu(zl[b,i] @ fc_l_W^T + fc_l_b) @ W[o] @ celu(zr[b,j] @ fc_r_W^T + fc_r_b) + bb[o]

with context-3 pairing:
  zl = [x_l, shift_fwd(x_l,1), shift_bwd(x_l,1)]   (192 features)
  zr = [x_l, shift_bwd(x_r,1), shift_fwd(x_r,1)]   (faithful torch-source bug: x_l first)

Shapes: B=2, N=512, n_in=64, H=128, n_out=8 -> out [2,512,512,8] f32.

Sharding: 8 cores = (b in {0,1}) x (j-chunk in {0..3} of 128 columns).
Each core computes out[b, :, j0:j0+128, :] = [512, 128, 8] = 2MB.

Per-core dataflow (contraction dims pre-transposed onto partitions host-side).
DMA completion receipts serialize (~1.8us each) per HWDGE ring, so inputs are
packed into THREE DMAs ordered by first use:
  D1 [128,1542] f32r = frT|xljT|xrhT|bl|br|flT|xlhT   (layer-1 operands+biases)
  D2 [128,1024] bf16 = per-o transposed bilinear_W
  D3 [128,512]  f32  = btile (bias bb tiled over (j,o))

  1. hrT[g,j] = celu(sum_c frT_c.T @ {xljT, xrhT +-1} + br)   3 f32r matmuls
  2. hlT[h,i] = celu(sum_c flT_c.T @ xlhT_shift_c + bl)       3 f32r matmuls
     celu(x) = max(x+b, min(exp(x+b) - 1, 0)) — exp overflow is clamped by
     the min. hlT/hrT are stored bf16 for the second-stage matmuls.
  3. v2[h,(j,o)] : per o: WT_o.T @ hrT (bf16) -> psum [h, o*128+j]; one
     strided cast per 4 o's into (j*8+o) layout.
  4. out[i,(j,o)] = hlT_chunk.T @ v2_half + btile  (8 bf16 matmuls, n=512).
     Evictions: first 3 chunks ACT copy + GpSimd bias-add, rest DVE fused.
  5. 8 output DMAs [128,512] (2KB/partition rows), split across both HWDGE
     rings (sync + scalar) to halve issue and receipt serialization.

walrus's per-instruction HW structs carry at most ONE sync wait; a post-pass
splits multi-wait instructions into single-wait EventSemaphore predecessors.
"""

import os

import numpy as np

import concourse.bass as bass
import concourse.mybir as mybir
import concourse.tile as tile
from concourse.bass_utils import run_bass_kernel_spmd

F32 = mybir.dt.float32
F32R = mybir.dt.float32r
BF16 = mybir.dt.bfloat16

B = 2
N = 512
NIN = 64
H = 128
O = 8
JC = 128  # j-chunk per core
N_CORES = 8

USE_F32R = os.environ.get("BK_F32R", "1") == "1"
USE_BF16 = os.environ.get("BK_BF16", "1") == "1"

# packed-D1 column offsets (f32r)
_FR0 = 0              # frT   [3*128]
_XLJ = 384            # xljT  [128]
_XRH = 512            # xrhT  [130]
_BL = 642             # fc_l_b [1]
_BR = 643             # fc_r_b [1]
_FL0 = 644            # flT   [3*128]
_XLH = 1028           # xlhT  [514]
_D1W = 1542


def build_nc():
    nc = bass.Bass("TRN2")
    dt_l1 = F32R if USE_F32R else F32
    dt_2 = BF16 if USE_BF16 else dt_l1

    D1 = nc.dram_tensor("D1", [128, _D1W], dt_l1, kind="ExternalInput")
    Wt = nc.dram_tensor("Wt", [128, O * H], dt_2, kind="ExternalInput")
    Ct = nc.dram_tensor("Ct", [128, 512], F32, kind="ExternalInput")
    out_d = nc.dram_tensor("out", [N, JC * O], F32, kind="ExternalOutput")

    with tile.TileContext(nc) as tc:
        with (
            tc.tile_pool(name="persist", bufs=1) as pp,
            tc.tile_pool(name="scratch", bufs=2) as sp,
            tc.tile_pool(name="outbuf", bufs=8) as op,
            tc.tile_pool(name="ps_l1", bufs=1, space="PSUM") as ps_l1,
            tc.tile_pool(name="ps_v", bufs=2, space="PSUM") as ps_v,
            tc.tile_pool(name="ps_main", bufs=4, space="PSUM") as ps_main,
        ):
            # Early no-dep ACT op so the lazy PWP activation-table load
            # (~1.3us) happens during the input-DMA wait.
            td = pp.tile([1, 2], F32, name="td")
            nc.vector.memset(td[0:1, 0:1], 0.0)
            nc.scalar.activation(td[0:1, 1:2], td[0:1, 0:1],
                                 mybir.ActivationFunctionType.Exp)

            D1_sb = pp.tile([128, _D1W], dt_l1, name="D1_sb")
            W_sb = pp.tile([128, O * H], dt_2, name="W_sb")
            Ct_sb = pp.tile([128, 512], F32, name="Ct_sb")
            nc.sync.dma_start(D1_sb[:], D1[:])
            nc.sync.dma_start(W_sb[:], Wt[:])
            nc.sync.dma_start(Ct_sb[:], Ct[:])

            bl_ap = D1_sb[:, _BL:_BL + 1].bitcast(F32)
            br_ap = D1_sb[:, _BR:_BR + 1].bitcast(F32)
            btile = Ct_sb[:]

            def celu_from_psum(psum, bias_ap, width, tag, split):
                """h = celu(psum + bias) = max(pre, min(exp(pre)-1, 0)).

                split=True computes pre on DVE (parallel with the ACT exp)
                to shorten the serial chain on the wide hl tensor."""
                pre = sp.tile([128, width], F32, name=f"pre_{tag}")
                e = sp.tile([128, width], F32, name=f"e_{tag}")
                h = pp.tile([128, width], dt_2, name=f"h_{tag}")
                if split:
                    nc.vector.tensor_scalar_add(pre[:], psum, bias_ap)
                else:
                    nc.scalar.activation(
                        pre[:], psum, mybir.ActivationFunctionType.Identity,
                        bias=bias_ap, scale=1.0,
                    )
                nc.scalar.activation(
                    e[:], psum, mybir.ActivationFunctionType.Exp,
                    bias=bias_ap, scale=1.0,
                )
                # s = min(e - 1, 0)   (fused; overflow in e is clamped here)
                nc.vector.tensor_scalar(
                    e[:], e[:], -1.0, 0.0,
                    mybir.AluOpType.add, mybir.AluOpType.min,
                )
                nc.vector.tensor_tensor(h[:], pre[:], e[:], mybir.AluOpType.max)
                return h

            # ---- layer 1: hrT [128(g), JC] ----
            ps_hr = ps_l1.tile([128, JC], F32, name="ps_hr")
            rhs_r = [
                D1_sb[:, _XLJ:_XLJ + JC],           # x_l[j]
                D1_sb[:, _XRH + 2:_XRH + 2 + JC],   # x_r[j+1] (bwd)
                D1_sb[:, _XRH:_XRH + JC],           # x_r[j-1] (fwd)
            ]
            for c in range(3):
                nc.tensor.matmul(
                    ps_hr[:], D1_sb[:, _FR0 + c * H:_FR0 + (c + 1) * H], rhs_r[c],
                    start=(c == 0), stop=(c == 2),
                )
            hrT = celu_from_psum(ps_hr[:], br_ap, JC, "hr", split=False)

            # ---- layer 1: hlT [128(h), N] ----
            ps_hl = ps_l1.tile([128, N], F32, name="ps_hl")
            rhs_l = [
                D1_sb[:, _XLH + 1:_XLH + 1 + N],    # x_l[i]
                D1_sb[:, _XLH:_XLH + N],            # x_l[i-1] (fwd)
                D1_sb[:, _XLH + 2:_XLH + 2 + N],    # x_l[i+1] (bwd)
            ]
            for c in range(3):
                nc.tensor.matmul(
                    ps_hl[:], D1_sb[:, _FL0 + c * H:_FL0 + (c + 1) * H], rhs_l[c],
                    start=(c == 0), stop=(c == 2),
                )
            hlT = celu_from_psum(ps_hl[:], bl_ap, N, "hl", split=True)

            # ---- v2[h, (j,o)] ----
            v2 = pp.tile([128, JC, O], dt_2, name="v2")
            for og in range(2):
                ps_vo = ps_v.tile([128, 512], F32, name="ps_vo")
                for ol in range(4):
                    o = og * 4 + ol
                    nc.tensor.matmul(
                        ps_vo[:, ol * JC:(ol + 1) * JC],
                        W_sb[:, o * H:(o + 1) * H], hrT[:],
                        start=True, stop=True,
                    )
                # strided cast: psum col (ol*128+j) -> v2[:, j, og*4+ol]
                cast_eng = nc.vector.tensor_copy if og == 0 else nc.scalar.copy
                cast_eng(
                    v2[:, :, og * 4:(og + 1) * 4],
                    ps_vo[:].rearrange("p (o j) -> p j o", o=4),
                )
            v2f = v2[:].rearrange("p j o -> p (j o)")

            # ---- main: out[i, (j,o)] ----
            for chunk in range(8):
                ic, half = chunk // 2, chunk % 2
                ps_m = ps_main.tile([128, 512], F32, name="ps_m")
                out_sb = op.tile([128, 512], F32, name="out_sb")
                nc.tensor.matmul(
                    ps_m[:],
                    hlT[:, ic * 128:(ic + 1) * 128],
                    v2f[:, half * 512:(half + 1) * 512],
                    start=True, stop=True,
                )
                if chunk < 3:
                    nc.scalar.copy(out_sb[:], ps_m[:])
                    nc.gpsimd.tensor_tensor(
                        out_sb[:], out_sb[:], btile, mybir.AluOpType.add)
                else:
                    nc.vector.tensor_tensor(
                        out_sb[:], ps_m[:], btile, mybir.AluOpType.add)
                dma_eng = nc.sync if chunk < 4 else nc.scalar
                dma_eng.dma_start(
                    out_d[ic * 128:(ic + 1) * 128,
                          half * 512:(half + 1) * 512],
                    out_sb[:])

    _legalize_waits(nc)
    return nc


def _legalize_waits(nc):
    """walrus's per-instruction HW structs carry at most ONE sync wait.
    Split any instruction with >1 on_wait into same-engine single-wait
    EventSemaphore predecessors (engine executes them in program order)."""
    n = 0
    for bb in nc.main_func.blocks:
        insts = list(bb.instructions)
        out = []
        for ins in insts:
            si = ins.sync_info
            waits = list(si.on_wait) if si and si.on_wait else []
            if len(waits) > 1:
                for w in waits[:-1]:
                    n += 1
                    out.append(mybir.InstEventSemaphore(
                        name=f"wait-split-{n}",
                        opcode="EventSemaphore",
                        engine=ins.engine,
                        ins=[], outs=[],
                        sync_info=mybir.SyncInfo(on_wait=[w], on_update=[]),
                    ))
                si.on_wait = [waits[-1]]
            out.append(ins)
        if n:
            bb.instructions = out
    return nc


_NC_CACHE = None


def _get_nc():
    global _NC_CACHE
    if _NC_CACHE is None:
        _NC_CACHE = build_nc()
    return _NC_CACHE


def _prep_core_inputs(x_l, x_r, fc_l_W, fc_l_b, fc_r_W, fc_r_b, bilinear_W, bilinear_b):
    """Host-side sharding: build the 8 per-core input dicts."""
    import ml_dtypes

    f32 = np.float32
    x_l = np.ascontiguousarray(x_l, f32)
    x_r = np.ascontiguousarray(x_r, f32)

    # fc weights: [H,192] -> transposed per 64-chunk, zero-padded to 128 rows
    def fcT(w):
        out = np.zeros((128, 3 * H), f32)
        for c in range(3):
            out[:NIN, c * H:(c + 1) * H] = w[:, c * NIN:(c + 1) * NIN].T
        return out

    flT = fcT(np.asarray(fc_l_W, f32))
    frT = fcT(np.asarray(fc_r_W, f32))
    # WT[g, o*H + h] = W[o, h, g]
    WT = np.ascontiguousarray(
        np.asarray(bilinear_W, f32).transpose(2, 0, 1).reshape(128, O * H))
    if USE_BF16:
        WT = WT.astype(ml_dtypes.bfloat16)
    Ct = np.ascontiguousarray(np.broadcast_to(
        np.tile(np.asarray(bilinear_b, f32), 512 // O), (128, 512)))

    D1c = np.zeros((128, _D1W), f32)
    D1c[:, _FR0:_FR0 + 3 * H] = frT
    D1c[:, _BL] = np.asarray(fc_l_b, f32)
    D1c[:, _BR] = np.asarray(fc_r_b, f32)
    D1c[:, _FL0:_FL0 + 3 * H] = flT

    in_maps = []
    for core in range(N_CORES):
        b, jg = core // 4, core % 4
        j0 = jg * JC
        D1 = D1c.copy()
        D1[:NIN, _XLJ:_XLJ + JC] = x_l[b, j0:j0 + JC].T
        # xrhT: col t = x_r[b, j0-1+t], zero-padded at global edges
        lo = max(j0 - 1, 0)
        hi = min(j0 + JC + 1, N)
        D1[:NIN, _XRH + lo - (j0 - 1):_XRH + hi - (j0 - 1)] = x_r[b, lo:hi].T
        # xlhT: col t = x_l[b, t-1], zeros at t=0 and t=N+1
        D1[:NIN, _XLH + 1:_XLH + 1 + N] = x_l[b].T
        in_maps.append({"D1": D1, "Wt": WT, "Ct": Ct})
    return in_maps


def _run(inputs, trace=False, **kw):
    nc = _get_nc()
    in_maps = _prep_core_inputs(**inputs)
    res = run_bass_kernel_spmd(
        nc, in_maps, core_ids=list(range(N_CORES)), trace=trace, **kw)
    out = np.empty((B, N, N, O), np.float32)
    for core in range(N_CORES):
        b, jg = core // 4, core % 4
        j0 = jg * JC
        out[b, :, j0:j0 + JC, :] = res.results[core]["out"].reshape(N, JC, O)
    return out, res


def kernel(**inputs):
    out, _ = _run(inputs, trace=False)
    return out
